# revision 14
# baseline (speedup 1.0000x reference)
"""TopK sparse autoencoder forward pass on 8 Trainium2 NeuronCores.

Math (reference):
    preact = (x - b_dec) @ W_enc.T + b_enc          # [B, F]
    top32 = exact per-row top-32 of relu(preact)
    x_hat = scatter(top32) @ W_dec.T + b_dec        # [B, D]

Strategy: data-parallel over batch rows (1024 rows/core, no collectives).
Per core:
  encode: bf16x3 matmul (PE) - xh*wh + xh*wl + xl*wh reproduces fp32 to
          ~2e-6 (the top-k boundary order-stat gaps are ~1e-2, so selection
          noise must stay ~1e-5 or row-level swaps blow the 2e-2 budget).
          f-block-outer loop; PSUM -> stage (Act) -> DRAM fp32 spill + DVE
          chunk-max into cm tiles.
  T1 (select), per 128-row tile: top-32 chunks by chunk-max via DVE
          max/max_index/match_replace rounds; gather those chunks from the
          spill (32 single-offset indirect DMAs - walrus mislowers
          multi-offset forms); 4 more rounds give exact top-32 vals+idx.
  T2 (decode): gather the 32 selected W_dec.T rows (fp16) per row and
          accumulate sum_c diag(vals[:,c]) @ G_c on the PE.
Pipelining: all engines are in-order, so T1 of group 0 is interleaved
between f-blocks of group 1's encode (keeps DVE/Pool busy without
queueing PE decode work ahead of encode). T2(g0) runs after encode ends;
only the last group's T1+T2 is exposed. Groups restream W_enc.
"""
import sys
sys.path.insert(0, '/opt/trn_rl_repo')

import numpy as np

B, D, F, K = 8192, 768, 16384, 32
N_CORES = 8
BC = B // N_CORES          # rows per core (1024)
NBT = BC // 128            # b-tiles per core (8)
NFB = F // 512             # f-blocks (32)
KD = D // 128              # contraction chunks (6)
CH = 32                    # topk chunk width
C = F // CH                # chunks per row
SH = CH.bit_length() - 1   # log2(CH)
NCH = K // 8               # rounds of 8 (4)
DH = 16                    # decode gather half size

ENC_MODE = "f32r"        # "fp32" | "f32r" | "bf16x3"
GROUPS = (6, 2)            # b-tile group sizes (sum = NBT)
WDEC_MODE = "slot"         # "slot" | "ant" (dma_gather)
STAGE_BUFS = 10

_cache = {}


def _fix_sync_waits(nc, maxw=1):
    """This container's walrus rejects >1 sync wait per instruction; split
    excess waits onto same-engine NoOps inserted just before."""
    import bass_rust
    import concourse.mybir as mybir
    ctr = 0
    for f in nc.m.functions:
        for bb in f.blocks:
            out, changed = [], False
            for inst in bb.instructions:
                si = inst.sync_info
                waits = list(si.on_wait) if si is not None else []
                if len(waits) > maxw:
                    changed = True
                    head, keep = waits[:-maxw], waits[-maxw:]
                    for i in range(0, len(head), maxw):
                        ctr += 1
                        nop = mybir.InstNoOp(
                            name=f"syncfix-nop-{id(nc)}-{ctr}", ins=[], outs=[])
                        nop.engine = inst.engine
                        nop.sync_info = bass_rust.SyncInfo(
                            on_wait=head[i:i + maxw], on_update=[])
                        out.append(nop)
                    si.on_wait = keep
                out.append(inst)
            if changed:
                bb.instructions = out


def _build(has_benc: bool, has_bdec: bool, repeat: int = 1):
    import concourse.bass as bass
    import concourse.mybir as mybir
    import concourse.tile as tile
    from concourse import library_config
    dt = mybir.dt
    Alu = mybir.AluOpType

    nc = bass.Bass("TRN2", target_bir_lowering=False, debug=False,
                   num_devices=N_CORES)

    if ENC_MODE == "bf16x3":
        xTh_d = nc.dram_tensor("xTh", [D, BC], dt.bfloat16, kind="ExternalInput")
        xTl_d = nc.dram_tensor("xTl", [D, BC], dt.bfloat16, kind="ExternalInput")
        wencTh_d = nc.dram_tensor("wencTh", [D, F], dt.bfloat16,
                                  kind="ExternalInput")
        wencTl_d = nc.dram_tensor("wencTl", [D, F], dt.bfloat16,
                                  kind="ExternalInput")
    else:
        _edt = dt.float32r if ENC_MODE == "f32r" else dt.float32
        xT_d = nc.dram_tensor("xT", [D, BC], _edt, kind="ExternalInput")
        wencT_d = nc.dram_tensor("wencT", [D, F], _edt,
                                 kind="ExternalInput")
    wdecT_d = nc.dram_tensor("wdecT16", [F, D], dt.float16, kind="ExternalInput")
    beff_d = nc.dram_tensor("beff", [1, F], dt.float32, kind="ExternalInput")
    bdec_d = nc.dram_tensor("bdec", [1, D], dt.float32, kind="ExternalInput")
    out_d = nc.dram_tensor("xhat", [BC, D], dt.float32, kind="ExternalOutput")
    # one spill tensor per b-tile group: gathers then never falsely conflict
    # (WAR) with the next group's spill writes in the dep tracker
    grp_of, loc_of, s = {}, {}, 0
    for gi, n in enumerate(GROUPS):
        for j in range(n):
            grp_of[s + j], loc_of[s + j] = gi, j
        s += n
    preact_ds = [nc.dram_tensor(f"preact_spill{gi}", [128 * n, F], dt.float32)
                 for gi, n in enumerate(GROUPS)]
    preact_flats = [p.ap().rearrange("b (c w) -> (b c) w", w=CH)
                    for p in preact_ds]

    def body(tc, pools):
        sb, sb1, stp, psA, psB, gp, wbp = pools

        # resident inputs
        if ENC_MODE == "bf16x3":
            xTh = sb1.tile([128, KD, BC], dt.bfloat16)
            nc.sync.dma_start(
                xTh[:], xTh_d.ap().rearrange("(po pi) b -> pi po b", pi=128))
            xTl = sb1.tile([128, KD, BC], dt.bfloat16)
            nc.sync.dma_start(
                xTl[:], xTl_d.ap().rearrange("(po pi) b -> pi po b", pi=128))
        else:
            xT = sb1.tile([128, KD, BC],
                          dt.float32r if ENC_MODE == "f32r" else dt.float32)
            nc.sync.dma_start(
                xT[:], xT_d.ap().rearrange("(po pi) b -> pi po b", pi=128))
        if has_benc:
            beff = sb1.tile([1, F], dt.float32)
            nc.sync.dma_start(beff[:], beff_d.ap())
        if has_bdec:
            bdec1 = sb1.tile([1, D], dt.float32)
            nc.sync.dma_start(bdec1[:], bdec_d.ap())
            bdec_bc = sb1.tile([128, D], dt.float32)
            nc.gpsimd.partition_broadcast(bdec_bc[:], bdec1[:])

        cms = [sb1.tile([128, C], dt.float32, name=f"cm{bt}") for bt in range(NBT)]
        stgq = [sb1.tile([128, 4, 512], dt.float32, name=f"stgq{bt}")
                for bt in range(NBT)]
        valss = [sb1.tile([128, K], dt.float32, name=f"vals{bt}")
                 for bt in range(NBT)]
        idxss = [sb1.tile([128, K], dt.uint32, name=f"idx{bt}")
                 for bt in range(NBT)]

        # iota constants (standard gpsimd library; must precede any
        # load_library(mlp))
        jiota = sb1.tile([128, K], dt.uint32)
        nc.gpsimd.iota(jiota[:], pattern=[[1, K]], base=0, channel_multiplier=0)
        # fp16 identity mask for building diag(vals) tiles on DVE
        iota_p = sb1.tile([128, 1], dt.uint32)
        nc.gpsimd.iota(iota_p[:], pattern=[[1, 1]], base=0, channel_multiplier=1)
        iota_f = sb1.tile([128, 128], dt.uint32)
        nc.gpsimd.iota(iota_f[:], pattern=[[1, 128]], base=0,
                       channel_multiplier=0)
        idmask = sb1.tile([128, 128], dt.float16)
        nc.vector.tensor_tensor(idmask[:],
                                iota_p[:, :1].to_broadcast([128, 128]),
                                iota_f[:], op=Alu.is_equal)
        rowoffs = [sb1.tile([128, 1], dt.uint32, name=f"ro{bt}")
                   for bt in range(NBT)]
        for bt in range(NBT):
            nc.gpsimd.iota(rowoffs[bt][:], pattern=[[1, 1]],
                           base=loc_of[bt] * 128 * C, channel_multiplier=C)
        if WDEC_MODE == "ant":
            nc.gpsimd.load_library(library_config.mlp)

        # ---------------- encode + spill + chunk-max ----------------
        if ENC_MODE == "bf16x3":
            wh_v = wencTh_d.ap().rearrange("(po pi) f -> pi po f", pi=128)
            wl_v = wencTl_d.ap().rearrange("(po pi) f -> pi po f", pi=128)
        else:
            wencT_v = wencT_d.ap().rearrange("(po pi) f -> pi po f", pi=128)

        def encode_fb(fb, bts):
            if ENC_MODE == "bf16x3":
                wbh = sb.tile([128, KD, 512], dt.bfloat16, tag="wbh")
                nc.sync.dma_start(wbh[:], wh_v[:, :, fb * 512:(fb + 1) * 512])
                wbl = sb.tile([128, KD, 512], dt.bfloat16, tag="wbl")
                nc.sync.dma_start(wbl[:], wl_v[:, :, fb * 512:(fb + 1) * 512])
            else:
                wb = wbp.tile([128, KD, 512],
                              dt.float32r if ENC_MODE == "f32r" else dt.float32,
                              tag="wb")
                nc.sync.dma_start(wb[:], wencT_v[:, :, fb * 512:(fb + 1) * 512])
            if has_benc:
                beff_bc = stp.tile([128, 512], dt.float32, tag="beffbc")
                nc.gpsimd.partition_broadcast(
                    beff_bc[:], beff[:, fb * 512:(fb + 1) * 512])
            for bt in bts:
                ps = psA.tile([128, 512], dt.float32, tag="encps")
                if ENC_MODE == "bf16x3":
                    bsl = slice(bt * 128, (bt + 1) * 128)
                    terms = [(xTh, wbh), (xTh, wbl), (xTl, wbh)]
                    n_mm = KD * len(terms)
                    i = 0
                    for k in range(KD):
                        for (a, w_) in terms:
                            nc.tensor.matmul(ps[:], lhsT=a[:, k, bsl],
                                             rhs=w_[:, k, :],
                                             start=(i == 0), stop=(i == n_mm - 1))
                            i += 1
                else:
                    for k in range(KD):
                        lhsT = xT[:, k, bt * 128:(bt + 1) * 128]
                        rhs = wb[:, k, :]
                        nc.tensor.matmul(ps[:], lhsT=lhsT, rhs=rhs,
                                         start=(k == 0), stop=(k == KD - 1))
                # evacuate PSUM via Act only (frees the bank fast; DVE
                # chunk-max reads the SBUF stage copy so encode never
                # stalls on DVE backlog). Four f-blocks accumulate in a
                # per-tile quad buffer; one 8KB-line spill DMA per quad.
                q = fb % 4
                stage = stgq[bt][:, q]
                if has_benc:
                    nc.vector.tensor_add(stage, ps[:], beff_bc[:])
                else:
                    nc.scalar.copy(stage, ps[:])
                nc.vector.tensor_reduce(
                    cms[bt][:, fb * (512 // CH):(fb + 1) * (512 // CH)],
                    stage.rearrange("p (c w) -> p c w", w=CH),
                    axis=mybir.AxisListType.X, op=Alu.max)
                if q == 3:
                    nc.sync.dma_start(
                        preact_ds[grp_of[bt]].ap()[
                            loc_of[bt] * 128:(loc_of[bt] + 1) * 128,
                            (fb - 3) * 512:(fb + 1) * 512],
                        stgq[bt][:].rearrange("p a b -> p (a b)"))

        # ---------------- T1: per-b-tile topk selection ----------------
        def tail_select(bt):
            # top-32 chunks by chunk max
            cmw = sb.tile([128, C], dt.float32, tag="cmw")
            nc.vector.tensor_copy(cmw[:], cms[bt][:])
            cm8 = sb.tile([128, 8], dt.float32, tag="cm8")
            chunkid = sb.tile([128, K], dt.uint32, tag="chunkid")
            for r in range(NCH):
                nc.vector.max(out=cm8[:], in_=cmw[:])
                nc.vector.max_index(out=chunkid[:, r * 8:(r + 1) * 8],
                                    in_max=cm8[:], in_values=cmw[:])
                if r != NCH - 1:
                    nc.vector.match_replace(out=cmw[:], in_to_replace=cm8[:],
                                            in_values=cmw[:], imm_value=-1e30)

            # gather the 32 chunks' contents from the DRAM spill
            off = sb.tile([128, K], dt.uint32, tag="off")
            nc.vector.tensor_tensor(off[:], chunkid[:],
                                    rowoffs[bt][:, :1].to_broadcast([128, K]),
                                    op=Alu.add)
            cand = sb.tile([128, K, CH], dt.float32, tag="cand")
            for j in range(K):
                nc.gpsimd.indirect_dma_start(
                    out=cand[:, j], out_offset=None,
                    in_=preact_flats[grp_of[bt]],
                    in_offset=bass.IndirectOffsetOnAxis(ap=off[:, j:j + 1],
                                                        axis=0))

            # exact top-32 of the candidates
            candf = cand[:].rearrange("p a b -> p (a b)")
            vals = valss[bt]
            pos = sb.tile([128, K], dt.uint32, tag="pos")
            for r in range(NCH):
                m8 = vals[:, r * 8:(r + 1) * 8]
                nc.vector.max(out=m8, in_=candf)
                nc.vector.max_index(out=pos[:, r * 8:(r + 1) * 8],
                                    in_max=m8, in_values=candf)
                if r != NCH - 1:
                    nc.vector.match_replace(out=candf, in_to_replace=m8,
                                            in_values=candf, imm_value=-1e30)

            # positions -> global feature indices:
            # idx = chunkid[p, pos>>SH]*CH + (pos&(CH-1)); chunkid lookup via
            # one-hot compare-multiply-reduce (no per-partition gather on HW)
            j32 = sb.tile([128, K], dt.uint32, tag="j32")
            nc.vector.tensor_scalar(j32[:], pos[:], SH, None,
                                    op0=Alu.logical_shift_right)
            l32 = sb.tile([128, K], dt.uint32, tag="l32")
            nc.vector.tensor_scalar(l32[:], pos[:], CH - 1, None,
                                    op0=Alu.bitwise_and)
            eq = sb.tile([128, K, K], dt.uint32, tag="eq")
            nc.vector.tensor_tensor(
                eq[:], j32[:, :, None].to_broadcast([128, K, K]),
                jiota[:, None, :].to_broadcast([128, K, K]), op=Alu.is_equal)
            nc.vector.tensor_tensor(
                eq[:], eq[:], chunkid[:, None, :].to_broadcast([128, K, K]),
                op=Alu.mult)
            cs32 = sb.tile([128, K], dt.uint32, tag="cs32")
            nc.vector.tensor_reduce(cs32[:], eq[:],
                                    axis=mybir.AxisListType.X, op=Alu.max)
            idx32 = idxss[bt]
            nc.vector.tensor_scalar(idx32[:], cs32[:], SH, None,
                                    op0=Alu.logical_shift_left)
            nc.vector.tensor_tensor(idx32[:], idx32[:], l32[:], op=Alu.add)

            # relu guard (rows with <32 positive preacts: extra top-k entries
            # are relu zeros in the reference; zero coefficients match it)
            nc.vector.tensor_scalar_max(vals[:], vals[:], 0.0)

            if WDEC_MODE == "ant":
                # build wrapped int16 index layout for dma_gather:
                # wrap[q, c*8+g] = idx[(g*16+q), c], replicated to all 8
                # 16-partition groups
                idx16 = sb.tile([128, K], dt.uint16, tag="idx16")
                nc.vector.tensor_copy(idx16[:], idx32[:])
                wrap = sb.tile([128, K * 8], dt.int16, tag="wrap",
                               name=f"wrap{bt}")
                wv = wrap[0:16, :].rearrange("q (c g) -> q c g", g=8)
                for g in range(8):
                    nc.sync.dma_start(
                        wv[:, :, g],
                        idx16[g * 16:(g + 1) * 16, :].bitcast(dt.int16))
                for g in range(1, 8):
                    nc.sync.dma_start(wrap[g * 16:(g + 1) * 16, :],
                                      wrap[0:16, :])
                tail_select.wraps[bt] = wrap

        tail_select.wraps = {}

        # ---------------- T2: per-b-tile compact decode ----------------
        def tail_decode(bt):
            vals, idx32 = valss[bt], idxss[bt]
            # diag[p, c, :] = vals[p, c] * (identity row p), built on DVE to
            # keep the Pool queue free for the gathers
            diag = sb.tile([128, K, 128], dt.float16, tag="diag")
            nc.vector.tensor_tensor(
                diag[:], vals[:, :, None].to_broadcast([128, K, 128]),
                idmask[:, None, :].to_broadcast([128, K, 128]), op=Alu.mult)

            pso = psB.tile([128, D], dt.float32, tag="decps")
            if WDEC_MODE == "ant":
                wrap = tail_select.wraps[bt]
                for h in range(K // DH):
                    g = sb.tile([128, DH, D], dt.float16, tag="g")
                    nc.gpsimd.dma_gather(
                        out_ap=g[:], in_ap=wdecT_d.ap(),
                        idxs_ap=wrap[:, h * (DH * 8):(h + 1) * (DH * 8)],
                        num_idxs=DH * 128, num_idxs_reg=DH * 128, elem_size=D)
                    for c in range(DH):
                        cc = h * DH + c
                        nc.tensor.matmul(pso[:, :512], lhsT=diag[:, cc, :],
                                         rhs=g[:, c, :512],
                                         start=(cc == 0), stop=(cc == K - 1))
                        nc.tensor.matmul(pso[:, 512:D], lhsT=diag[:, cc, :],
                                         rhs=g[:, c, 512:D],
                                         start=(cc == 0), stop=(cc == K - 1))
            else:
                for c in range(K):
                    g = gp.tile([128, D], dt.float16, tag="g")
                    nc.gpsimd.indirect_dma_start(
                        out=g[:], out_offset=None,
                        in_=wdecT_d.ap(),
                        in_offset=bass.IndirectOffsetOnAxis(
                            ap=idx32[:, c:c + 1], axis=0))
                    nc.tensor.matmul(pso[:, :512], lhsT=diag[:, c, :],
                                     rhs=g[:, :512],
                                     start=(c == 0), stop=(c == K - 1))
                    nc.tensor.matmul(pso[:, 512:D], lhsT=diag[:, c, :],
                                     rhs=g[:, 512:D],
                                     start=(c == 0), stop=(c == K - 1))
            osb = stp.tile([128, D], dt.float32, tag="osb")
            if has_bdec:
                nc.vector.tensor_add(osb[:], pso[:], bdec_bc[:])
            else:
                nc.scalar.copy(osb[:], pso[:])
            nc.sync.dma_start(out_d.ap()[bt * 128:(bt + 1) * 128, :], osb[:])

        # ---------------- schedule ----------------
        groups, s = [], 0
        for n in GROUPS:
            groups.append(list(range(s, s + n)))
            s += n
        assert s == NBT

        prev = []
        for bts in groups:
            # encode this group, interleaving the previous group's T1 tails
            # between f-blocks (spreads DVE/Pool work; PE stream stays pure
            # encode)
            if prev:
                step = max(1, NFB // (len(prev) + 1))
                t1_at = {step * (i + 1): prev[i] for i in range(len(prev))}
            else:
                t1_at = {}
            for fb in range(NFB):
                encode_fb(fb, bts)
                if fb in t1_at:
                    tail_select(t1_at[fb])
            for bt in prev:
                tail_decode(bt)
            prev = bts
        for bt in prev:
            tail_select(bt)
        for bt in prev:
            tail_decode(bt)

    from contextlib import ExitStack
    with tile.TileContext(nc) as tc:
        with ExitStack() as ctx:
            pools = (
                ctx.enter_context(tc.tile_pool(name="sb", bufs=2)),
                ctx.enter_context(tc.tile_pool(name="sb1", bufs=1)),
                ctx.enter_context(tc.tile_pool(name="stage", bufs=4)),
                ctx.enter_context(tc.tile_pool(name="psA", bufs=6, space="PSUM")),
                ctx.enter_context(tc.tile_pool(name="psB", bufs=1, space="PSUM")),
                ctx.enter_context(tc.tile_pool(name="gpool", bufs=6)),
                ctx.enter_context(tc.tile_pool(name="wbpool", bufs=3)),
            )
            if repeat == 1:
                body(tc, pools)
            else:
                with tc.For_i(0, repeat, 1):
                    body(tc, pools)

    _fix_sync_waits(nc)
    return nc


def _get_runner(has_benc, has_bdec, repeat=1):
    key = (has_benc, has_bdec, repeat, ENC_MODE, GROUPS, CH, DH, WDEC_MODE)
    if key in _cache:
        return _cache[key]
    import jax
    from jax.sharding import Mesh, PartitionSpec
    from jax.experimental.shard_map import shard_map
    import concourse.mybir as mybir
    from concourse import bass2jax
    from concourse.bass2jax import _bass_exec_p, install_neuronx_cc_hook

    nc = _build(has_benc, has_bdec, repeat)
    install_neuronx_cc_hook()

    partition_name = (nc.partition_id_tensor.name
                      if nc.partition_id_tensor else None)
    in_names, out_names, out_avals, zero_outs = [], [], [], []
    for alloc in nc.m.functions[0].allocations:
        if not isinstance(alloc, mybir.MemoryLocationSet):
            continue
        name = alloc.memorylocations[0].name
        if alloc.kind == "ExternalInput":
            if name != partition_name:
                in_names.append(name)
        elif alloc.kind == "ExternalOutput":
            shape = tuple(alloc.tensor_shape)
            dtype = mybir.dt.np(alloc.dtype)
            out_names.append(name)
            out_avals.append(jax.core.ShapedArray(shape, dtype))
            zero_outs.append(np.zeros(shape, dtype))
    n_params = len(in_names)
    all_in = in_names + out_names
    if partition_name is not None:
        all_in = all_in + [partition_name]

    def _bodyfn(*args):
        operands = list(args)
        if partition_name is not None:
            operands.append(bass2jax.partition_id_tensor())
        outs = _bass_exec_p.bind(
            *operands, out_avals=tuple(out_avals), in_names=tuple(all_in),
            out_names=tuple(out_names), lowering_input_output_aliases=(),
            sim_require_finite=True, sim_require_nnan=True, nc=nc)
        return tuple(outs)

    try:
        devices = jax.devices("axon")[:N_CORES]
    except Exception:
        devices = jax.devices()[:N_CORES]
    mesh = Mesh(np.asarray(devices), ("core",))
    n_outs = len(out_names)
    fn = jax.jit(
        shard_map(_bodyfn, mesh=mesh,
                  in_specs=(PartitionSpec("core"),) * (n_params + n_outs),
                  out_specs=(PartitionSpec("core"),) * n_outs,
                  check_rep=False),
        keep_unused=True)
    sharding = jax.sharding.NamedSharding(mesh, PartitionSpec("core"))
    r = {"fn": fn, "in_names": in_names, "out_names": out_names,
         "zero_outs": zero_outs, "nc": nc, "sharding": sharding}
    _cache[key] = r
    return r


def _prep_host(x, W_enc, b_enc, W_dec, b_dec):
    x_eff = x - b_dec[None, :]
    xT_full = np.ascontiguousarray(x_eff.T, dtype=np.float32)      # [D, B]
    wencT = np.ascontiguousarray(W_enc.T, dtype=np.float32)        # [D, F]
    wdecT16 = np.ascontiguousarray(W_dec.T, dtype=np.float16)      # [F, D]
    beff = (b_enc.astype(np.float64)
            - W_enc.astype(np.float64) @ b_dec.astype(np.float64))
    beff = beff.astype(np.float32)[None, :]                        # [1, F]
    bdec = b_dec.astype(np.float32)[None, :]                       # [1, D]
    return xT_full, wencT, wdecT16, beff, bdec


def kernel(x, W_enc, b_enc, W_dec, b_dec, _repeat=1, _timeit=False):
    x = np.asarray(x, np.float32)
    W_enc = np.asarray(W_enc, np.float32)
    b_enc = np.asarray(b_enc, np.float32)
    W_dec = np.asarray(W_dec, np.float32)
    b_dec = np.asarray(b_dec, np.float32)
    xT_full, wencT, wdecT16, beff, bdec = _prep_host(x, W_enc, b_enc, W_dec, b_dec)
    has_benc = bool(np.any(beff))
    has_bdec = bool(np.any(b_dec))
    r = _get_runner(has_benc, has_bdec, _repeat)

    per_core = {
        "wdecT16": [wdecT16] * N_CORES,
        "beff": [beff] * N_CORES,
        "bdec": [bdec] * N_CORES,
    }
    if ENC_MODE == "bf16x3":
        import ml_dtypes
        bf16 = ml_dtypes.bfloat16
        xTh = xT_full.astype(bf16)
        xTl = (xT_full - xTh.astype(np.float32)).astype(bf16)
        wh = wencT.astype(bf16)
        wl = (wencT - wh.astype(np.float32)).astype(bf16)
        per_core["xTh"] = [np.ascontiguousarray(xTh[:, c * BC:(c + 1) * BC])
                           for c in range(N_CORES)]
        per_core["xTl"] = [np.ascontiguousarray(xTl[:, c * BC:(c + 1) * BC])
                           for c in range(N_CORES)]
        per_core["wencTh"] = [wh] * N_CORES
        per_core["wencTl"] = [wl] * N_CORES
    else:
        per_core["xT"] = [np.ascontiguousarray(xT_full[:, c * BC:(c + 1) * BC])
                          for c in range(N_CORES)]
        per_core["wencT"] = [wencT] * N_CORES
    args = [np.concatenate(per_core[name], axis=0) for name in r["in_names"]]
    args += [np.concatenate([z] * N_CORES, axis=0) for z in r["zero_outs"]]

    import jax, time
    dev_args = [jax.device_put(a, r["sharding"]) for a in args]
    kernel.last_dev_args = dev_args
    kernel.last_runner = r
    outs = r["fn"](*dev_args)
    jax.block_until_ready(outs)
    if _timeit:
        times = []
        for _ in range(_timeit if isinstance(_timeit, int) and _timeit > 1 else 8):
            t0 = time.perf_counter()
            outs = r["fn"](*dev_args)
            jax.block_until_ready(outs)
            times.append(time.perf_counter() - t0)
        kernel.last_times = times

    xhat = np.asarray(outs[r["out_names"].index("xhat")])  # [B, D] concat
    return xhat.astype(np.float32)


# revision 16
# speedup vs baseline: 1.1815x; 1.1815x over previous
"""TopK sparse autoencoder forward pass on 8 Trainium2 NeuronCores.

Math (reference):
    preact = (x - b_dec) @ W_enc.T + b_enc          # [B, F]
    top32 = exact per-row top-32 of relu(preact)
    x_hat = scatter(top32) @ W_dec.T + b_dec        # [B, D]

Strategy: data-parallel over batch rows (1024 rows/core, no collectives).
Per core:
  encode: bf16x3 matmul (PE) - xh*wh + xh*wl + xl*wh reproduces fp32 to
          ~2e-6 (the top-k boundary order-stat gaps are ~1e-2, so selection
          noise must stay ~1e-5 or row-level swaps blow the 2e-2 budget).
          f-block-outer loop; PSUM -> stage (Act) -> DRAM fp32 spill + DVE
          chunk-max into cm tiles.
  T1 (select), per 128-row tile: top-32 chunks by chunk-max via DVE
          max/max_index/match_replace rounds; gather those chunks from the
          spill (32 single-offset indirect DMAs - walrus mislowers
          multi-offset forms); 4 more rounds give exact top-32 vals+idx.
  T2 (decode): gather the 32 selected W_dec.T rows (fp16) per row and
          accumulate sum_c diag(vals[:,c]) @ G_c on the PE.
Pipelining: all engines are in-order, so T1 of group 0 is interleaved
between f-blocks of group 1's encode (keeps DVE/Pool busy without
queueing PE decode work ahead of encode). T2(g0) runs after encode ends;
only the last group's T1+T2 is exposed. Groups restream W_enc.
"""
import sys
sys.path.insert(0, '/opt/trn_rl_repo')

import numpy as np

B, D, F, K = 8192, 768, 16384, 32
N_CORES = 8
BC = B // N_CORES          # rows per core (1024)
NBT = BC // 128            # b-tiles per core (8)
NFB = F // 512             # f-blocks (32)
KD = D // 128              # contraction chunks (6)
CH = 32                    # topk chunk width
C = F // CH                # chunks per row
SH = CH.bit_length() - 1   # log2(CH)
NCH = K // 8               # rounds of 8 (4)
DH = 16                    # decode gather half size

ENC_MODE = "f32r"        # "fp32" | "f32r" | "bf16x3"
GROUPS = (6, 2)            # b-tile group sizes (sum = NBT)
WDEC_MODE = "slot"         # "slot" | "ant" (dma_gather)
STAGE_BUFS = 10

_cache = {}


def _fix_sync_waits(nc, maxw=1):
    """This container's walrus rejects >1 sync wait per instruction; split
    excess waits onto same-engine NoOps inserted just before."""
    import bass_rust
    import concourse.mybir as mybir
    ctr = 0
    for f in nc.m.functions:
        for bb in f.blocks:
            out, changed = [], False
            for inst in bb.instructions:
                si = inst.sync_info
                waits = list(si.on_wait) if si is not None else []
                if len(waits) > maxw:
                    changed = True
                    head, keep = waits[:-maxw], waits[-maxw:]
                    for i in range(0, len(head), maxw):
                        ctr += 1
                        nop = mybir.InstNoOp(
                            name=f"syncfix-nop-{id(nc)}-{ctr}", ins=[], outs=[])
                        nop.engine = inst.engine
                        nop.sync_info = bass_rust.SyncInfo(
                            on_wait=head[i:i + maxw], on_update=[])
                        out.append(nop)
                    si.on_wait = keep
                out.append(inst)
            if changed:
                bb.instructions = out


def _build(has_benc: bool, has_bdec: bool, repeat: int = 1):
    import concourse.bass as bass
    import concourse.mybir as mybir
    import concourse.tile as tile
    from concourse import library_config
    dt = mybir.dt
    Alu = mybir.AluOpType

    nc = bass.Bass("TRN2", target_bir_lowering=False, debug=False,
                   num_devices=N_CORES)

    if ENC_MODE == "bf16x3":
        xTh_d = nc.dram_tensor("xTh", [D, BC], dt.bfloat16, kind="ExternalInput")
        xTl_d = nc.dram_tensor("xTl", [D, BC], dt.bfloat16, kind="ExternalInput")
        wencTh_d = nc.dram_tensor("wencTh", [D, F], dt.bfloat16,
                                  kind="ExternalInput")
        wencTl_d = nc.dram_tensor("wencTl", [D, F], dt.bfloat16,
                                  kind="ExternalInput")
    else:
        _edt = dt.float32r if ENC_MODE == "f32r" else dt.float32
        xT_d = nc.dram_tensor("xT", [D, BC], _edt, kind="ExternalInput")
        wencT_d = nc.dram_tensor("wencT", [D, F], _edt,
                                 kind="ExternalInput")
    wdecT_d = nc.dram_tensor("wdecT16", [F, D], dt.float16, kind="ExternalInput")
    beff_d = nc.dram_tensor("beff", [1, F], dt.float32, kind="ExternalInput")
    bdec_d = nc.dram_tensor("bdec", [1, D], dt.float32, kind="ExternalInput")
    out_d = nc.dram_tensor("xhat", [BC, D], dt.float32, kind="ExternalOutput")
    # one spill tensor per b-tile group: gathers then never falsely conflict
    # (WAR) with the next group's spill writes in the dep tracker
    grp_of, loc_of, s = {}, {}, 0
    for gi, n in enumerate(GROUPS):
        for j in range(n):
            grp_of[s + j], loc_of[s + j] = gi, j
        s += n
    preact_ds = [nc.dram_tensor(f"preact_spill{gi}", [128 * n, F], dt.float32)
                 for gi, n in enumerate(GROUPS)]
    preact_flats = [p.ap().rearrange("b (c w) -> (b c) w", w=CH)
                    for p in preact_ds]

    def body(tc, pools):
        sb, sb1, stp, psA, psB, gp, wbp = pools

        # resident inputs
        if ENC_MODE == "bf16x3":
            xTh = sb1.tile([128, KD, BC], dt.bfloat16)
            nc.sync.dma_start(
                xTh[:], xTh_d.ap().rearrange("(po pi) b -> pi po b", pi=128))
            xTl = sb1.tile([128, KD, BC], dt.bfloat16)
            nc.sync.dma_start(
                xTl[:], xTl_d.ap().rearrange("(po pi) b -> pi po b", pi=128))
        else:
            xT = sb1.tile([128, KD, BC],
                          dt.float32r if ENC_MODE == "f32r" else dt.float32)
            nc.sync.dma_start(
                xT[:], xT_d.ap().rearrange("(po pi) b -> pi po b", pi=128))
        if has_benc:
            beff = sb1.tile([1, F], dt.float32)
            nc.sync.dma_start(beff[:], beff_d.ap())
        if has_bdec:
            bdec1 = sb1.tile([1, D], dt.float32)
            nc.sync.dma_start(bdec1[:], bdec_d.ap())
            bdec_bc = sb1.tile([128, D], dt.float32)
            nc.gpsimd.partition_broadcast(bdec_bc[:], bdec1[:])

        cms = [sb1.tile([128, C], dt.float32, name=f"cm{bt}") for bt in range(NBT)]
        stgq = [[sb1.tile([128, 2, 512], dt.float32, name=f"stgq{bt}_{j}")
                 for j in range(2)] for bt in range(NBT)]
        valss = [sb1.tile([128, K], dt.float32, name=f"vals{bt}")
                 for bt in range(NBT)]
        idxss = [sb1.tile([128, K], dt.uint32, name=f"idx{bt}")
                 for bt in range(NBT)]

        # iota constants (standard gpsimd library; must precede any
        # load_library(mlp))
        jiota = sb1.tile([128, K], dt.uint32)
        nc.gpsimd.iota(jiota[:], pattern=[[1, K]], base=0, channel_multiplier=0)
        # fp16 identity mask for building diag(vals) tiles on DVE
        iota_p = sb1.tile([128, 1], dt.uint32)
        nc.gpsimd.iota(iota_p[:], pattern=[[1, 1]], base=0, channel_multiplier=1)
        iota_f = sb1.tile([128, 128], dt.uint32)
        nc.gpsimd.iota(iota_f[:], pattern=[[1, 128]], base=0,
                       channel_multiplier=0)
        idmask = sb1.tile([128, 128], dt.float16)
        nc.vector.tensor_tensor(idmask[:],
                                iota_p[:, :1].to_broadcast([128, 128]),
                                iota_f[:], op=Alu.is_equal)
        rowoffs = [sb1.tile([128, 1], dt.uint32, name=f"ro{bt}")
                   for bt in range(NBT)]
        for bt in range(NBT):
            nc.gpsimd.iota(rowoffs[bt][:], pattern=[[1, 1]],
                           base=loc_of[bt] * 128 * C, channel_multiplier=C)
        if WDEC_MODE == "ant":
            nc.gpsimd.load_library(library_config.mlp)

        # ---------------- encode + spill + chunk-max ----------------
        if ENC_MODE == "bf16x3":
            wh_v = wencTh_d.ap().rearrange("(po pi) f -> pi po f", pi=128)
            wl_v = wencTl_d.ap().rearrange("(po pi) f -> pi po f", pi=128)
        else:
            wencT_v = wencT_d.ap().rearrange("(po pi) f -> pi po f", pi=128)

        def encode_fb(fb, bts):
            if ENC_MODE == "bf16x3":
                wbh = sb.tile([128, KD, 512], dt.bfloat16, tag="wbh")
                nc.sync.dma_start(wbh[:], wh_v[:, :, fb * 512:(fb + 1) * 512])
                wbl = sb.tile([128, KD, 512], dt.bfloat16, tag="wbl")
                nc.sync.dma_start(wbl[:], wl_v[:, :, fb * 512:(fb + 1) * 512])
            else:
                wb = wbp.tile([128, KD, 512],
                              dt.float32r if ENC_MODE == "f32r" else dt.float32,
                              tag="wb")
                nc.sync.dma_start(wb[:], wencT_v[:, :, fb * 512:(fb + 1) * 512])
            if has_benc:
                beff_bc = stp.tile([128, 512], dt.float32, tag="beffbc")
                nc.gpsimd.partition_broadcast(
                    beff_bc[:], beff[:, fb * 512:(fb + 1) * 512])
            for bt in bts:
                ps = psA.tile([128, 512], dt.float32, tag="encps")
                if ENC_MODE == "bf16x3":
                    bsl = slice(bt * 128, (bt + 1) * 128)
                    terms = [(xTh, wbh), (xTh, wbl), (xTl, wbh)]
                    n_mm = KD * len(terms)
                    i = 0
                    for k in range(KD):
                        for (a, w_) in terms:
                            nc.tensor.matmul(ps[:], lhsT=a[:, k, bsl],
                                             rhs=w_[:, k, :],
                                             start=(i == 0), stop=(i == n_mm - 1))
                            i += 1
                else:
                    for k in range(KD):
                        lhsT = xT[:, k, bt * 128:(bt + 1) * 128]
                        rhs = wb[:, k, :]
                        nc.tensor.matmul(ps[:], lhsT=lhsT, rhs=rhs,
                                         start=(k == 0), stop=(k == KD - 1))
                # evacuate PSUM via Act only (frees the bank fast; DVE
                # chunk-max reads the SBUF stage copy so encode never
                # stalls on DVE backlog). Four f-blocks accumulate in a
                # per-tile quad buffer; one 8KB-line spill DMA per quad.
                q = fb % 2
                sq = stgq[bt][(fb // 2) % 2]
                stage = sq[:, q]
                if has_benc:
                    nc.vector.tensor_add(stage, ps[:], beff_bc[:])
                else:
                    nc.scalar.copy(stage, ps[:])
                nc.vector.tensor_reduce(
                    cms[bt][:, fb * (512 // CH):(fb + 1) * (512 // CH)],
                    stage.rearrange("p (c w) -> p c w", w=CH),
                    axis=mybir.AxisListType.X, op=Alu.max)
                if q == 1:
                    nc.sync.dma_start(
                        preact_ds[grp_of[bt]].ap()[
                            loc_of[bt] * 128:(loc_of[bt] + 1) * 128,
                            (fb - 1) * 512:(fb + 1) * 512],
                        sq[:].rearrange("p a b -> p (a b)"))

        # ---------------- T1: per-b-tile topk selection ----------------
        def tail_select(bt):
            # top-32 chunks by chunk max
            cmw = sb.tile([128, C], dt.float32, tag="cmw")
            nc.vector.tensor_copy(cmw[:], cms[bt][:])
            cm8 = sb.tile([128, 8], dt.float32, tag="cm8")
            chunkid = sb.tile([128, K], dt.uint32, tag="chunkid")
            for r in range(NCH):
                nc.vector.max(out=cm8[:], in_=cmw[:])
                nc.vector.max_index(out=chunkid[:, r * 8:(r + 1) * 8],
                                    in_max=cm8[:], in_values=cmw[:])
                if r != NCH - 1:
                    nc.vector.match_replace(out=cmw[:], in_to_replace=cm8[:],
                                            in_values=cmw[:], imm_value=-1e30)

            # gather the 32 chunks' contents from the DRAM spill
            off = sb.tile([128, K], dt.uint32, tag="off")
            nc.vector.tensor_tensor(off[:], chunkid[:],
                                    rowoffs[bt][:, :1].to_broadcast([128, K]),
                                    op=Alu.add)
            cand = sb.tile([128, K, CH], dt.float32, tag="cand")
            for j in range(K):
                nc.gpsimd.indirect_dma_start(
                    out=cand[:, j], out_offset=None,
                    in_=preact_flats[grp_of[bt]],
                    in_offset=bass.IndirectOffsetOnAxis(ap=off[:, j:j + 1],
                                                        axis=0))

            # exact top-32 of the candidates
            candf = cand[:].rearrange("p a b -> p (a b)")
            vals = valss[bt]
            pos = sb.tile([128, K], dt.uint32, tag="pos")
            for r in range(NCH):
                m8 = vals[:, r * 8:(r + 1) * 8]
                nc.vector.max(out=m8, in_=candf)
                nc.vector.max_index(out=pos[:, r * 8:(r + 1) * 8],
                                    in_max=m8, in_values=candf)
                if r != NCH - 1:
                    nc.vector.match_replace(out=candf, in_to_replace=m8,
                                            in_values=candf, imm_value=-1e30)

            # positions -> global feature indices:
            # idx = chunkid[p, pos>>SH]*CH + (pos&(CH-1)); chunkid lookup via
            # one-hot compare-multiply-reduce (no per-partition gather on HW)
            j32 = sb.tile([128, K], dt.uint32, tag="j32")
            nc.vector.tensor_scalar(j32[:], pos[:], SH, None,
                                    op0=Alu.logical_shift_right)
            l32 = sb.tile([128, K], dt.uint32, tag="l32")
            nc.vector.tensor_scalar(l32[:], pos[:], CH - 1, None,
                                    op0=Alu.bitwise_and)
            eq = sb.tile([128, K, K], dt.uint32, tag="eq")
            nc.vector.tensor_tensor(
                eq[:], j32[:, :, None].to_broadcast([128, K, K]),
                jiota[:, None, :].to_broadcast([128, K, K]), op=Alu.is_equal)
            nc.vector.tensor_tensor(
                eq[:], eq[:], chunkid[:, None, :].to_broadcast([128, K, K]),
                op=Alu.mult)
            cs32 = sb.tile([128, K], dt.uint32, tag="cs32")
            nc.vector.tensor_reduce(cs32[:], eq[:],
                                    axis=mybir.AxisListType.X, op=Alu.max)
            idx32 = idxss[bt]
            nc.vector.tensor_scalar(idx32[:], cs32[:], SH, None,
                                    op0=Alu.logical_shift_left)
            nc.vector.tensor_tensor(idx32[:], idx32[:], l32[:], op=Alu.add)

            # relu guard (rows with <32 positive preacts: extra top-k entries
            # are relu zeros in the reference; zero coefficients match it)
            nc.vector.tensor_scalar_max(vals[:], vals[:], 0.0)

            if WDEC_MODE == "ant":
                # build wrapped int16 index layout for dma_gather:
                # wrap[q, c*8+g] = idx[(g*16+q), c], replicated to all 8
                # 16-partition groups
                idx16 = sb.tile([128, K], dt.uint16, tag="idx16")
                nc.vector.tensor_copy(idx16[:], idx32[:])
                wrap = sb.tile([128, K * 8], dt.int16, tag="wrap",
                               name=f"wrap{bt}")
                wv = wrap[0:16, :].rearrange("q (c g) -> q c g", g=8)
                for g in range(8):
                    nc.sync.dma_start(
                        wv[:, :, g],
                        idx16[g * 16:(g + 1) * 16, :].bitcast(dt.int16))
                for g in range(1, 8):
                    nc.sync.dma_start(wrap[g * 16:(g + 1) * 16, :],
                                      wrap[0:16, :])
                tail_select.wraps[bt] = wrap

        tail_select.wraps = {}

        # ---------------- T2: per-b-tile compact decode ----------------
        def tail_decode(bt):
            vals, idx32 = valss[bt], idxss[bt]
            # diag[p, c, :] = vals[p, c] * (identity row p), built on DVE to
            # keep the Pool queue free for the gathers
            diag = sb.tile([128, K, 128], dt.float16, tag="diag")
            nc.vector.tensor_tensor(
                diag[:], vals[:, :, None].to_broadcast([128, K, 128]),
                idmask[:, None, :].to_broadcast([128, K, 128]), op=Alu.mult)

            pso = psB.tile([128, D], dt.float32, tag="decps")
            if WDEC_MODE == "ant":
                wrap = tail_select.wraps[bt]
                for h in range(K // DH):
                    g = sb.tile([128, DH, D], dt.float16, tag="g")
                    nc.gpsimd.dma_gather(
                        out_ap=g[:], in_ap=wdecT_d.ap(),
                        idxs_ap=wrap[:, h * (DH * 8):(h + 1) * (DH * 8)],
                        num_idxs=DH * 128, num_idxs_reg=DH * 128, elem_size=D)
                    for c in range(DH):
                        cc = h * DH + c
                        nc.tensor.matmul(pso[:, :512], lhsT=diag[:, cc, :],
                                         rhs=g[:, c, :512],
                                         start=(cc == 0), stop=(cc == K - 1))
                        nc.tensor.matmul(pso[:, 512:D], lhsT=diag[:, cc, :],
                                         rhs=g[:, c, 512:D],
                                         start=(cc == 0), stop=(cc == K - 1))
            else:
                for c in range(K):
                    g = gp.tile([128, D], dt.float16, tag="g")
                    nc.gpsimd.indirect_dma_start(
                        out=g[:], out_offset=None,
                        in_=wdecT_d.ap(),
                        in_offset=bass.IndirectOffsetOnAxis(
                            ap=idx32[:, c:c + 1], axis=0))
                    nc.tensor.matmul(pso[:, :512], lhsT=diag[:, c, :],
                                     rhs=g[:, :512],
                                     start=(c == 0), stop=(c == K - 1))
                    nc.tensor.matmul(pso[:, 512:D], lhsT=diag[:, c, :],
                                     rhs=g[:, 512:D],
                                     start=(c == 0), stop=(c == K - 1))
            osb = stp.tile([128, D], dt.float32, tag="osb")
            if has_bdec:
                nc.vector.tensor_add(osb[:], pso[:], bdec_bc[:])
            else:
                nc.scalar.copy(osb[:], pso[:])
            nc.sync.dma_start(out_d.ap()[bt * 128:(bt + 1) * 128, :], osb[:])

        # ---------------- schedule ----------------
        groups, s = [], 0
        for n in GROUPS:
            groups.append(list(range(s, s + n)))
            s += n
        assert s == NBT

        prev = []
        for bts in groups:
            # encode this group, interleaving the previous group's T1 tails
            # between f-blocks (spreads DVE/Pool work; PE stream stays pure
            # encode)
            if prev:
                step = max(1, NFB // (len(prev) + 1))
                t1_at = {step * (i + 1): prev[i] for i in range(len(prev))}
            else:
                t1_at = {}
            for fb in range(NFB):
                encode_fb(fb, bts)
                if fb in t1_at:
                    tail_select(t1_at[fb])
            for bt in prev:
                tail_decode(bt)
            prev = bts
        for bt in prev:
            tail_select(bt)
        for bt in prev:
            tail_decode(bt)

    from contextlib import ExitStack
    with tile.TileContext(nc) as tc:
        with ExitStack() as ctx:
            pools = (
                ctx.enter_context(tc.tile_pool(name="sb", bufs=2)),
                ctx.enter_context(tc.tile_pool(name="sb1", bufs=1)),
                ctx.enter_context(tc.tile_pool(name="stage", bufs=4)),
                ctx.enter_context(tc.tile_pool(name="psA", bufs=6, space="PSUM")),
                ctx.enter_context(tc.tile_pool(name="psB", bufs=1, space="PSUM")),
                ctx.enter_context(tc.tile_pool(name="gpool", bufs=10)),
                ctx.enter_context(tc.tile_pool(name="wbpool", bufs=3)),
            )
            if repeat == 1:
                body(tc, pools)
            else:
                with tc.For_i(0, repeat, 1):
                    body(tc, pools)

    _fix_sync_waits(nc)
    return nc


def _get_runner(has_benc, has_bdec, repeat=1):
    key = (has_benc, has_bdec, repeat, ENC_MODE, GROUPS, CH, DH, WDEC_MODE)
    if key in _cache:
        return _cache[key]
    import jax
    from jax.sharding import Mesh, PartitionSpec
    from jax.experimental.shard_map import shard_map
    import concourse.mybir as mybir
    from concourse import bass2jax
    from concourse.bass2jax import _bass_exec_p, install_neuronx_cc_hook

    nc = _build(has_benc, has_bdec, repeat)
    install_neuronx_cc_hook()

    partition_name = (nc.partition_id_tensor.name
                      if nc.partition_id_tensor else None)
    in_names, out_names, out_avals, zero_outs = [], [], [], []
    for alloc in nc.m.functions[0].allocations:
        if not isinstance(alloc, mybir.MemoryLocationSet):
            continue
        name = alloc.memorylocations[0].name
        if alloc.kind == "ExternalInput":
            if name != partition_name:
                in_names.append(name)
        elif alloc.kind == "ExternalOutput":
            shape = tuple(alloc.tensor_shape)
            dtype = mybir.dt.np(alloc.dtype)
            out_names.append(name)
            out_avals.append(jax.core.ShapedArray(shape, dtype))
            zero_outs.append(np.zeros(shape, dtype))
    n_params = len(in_names)
    all_in = in_names + out_names
    if partition_name is not None:
        all_in = all_in + [partition_name]

    def _bodyfn(*args):
        operands = list(args)
        if partition_name is not None:
            operands.append(bass2jax.partition_id_tensor())
        outs = _bass_exec_p.bind(
            *operands, out_avals=tuple(out_avals), in_names=tuple(all_in),
            out_names=tuple(out_names), lowering_input_output_aliases=(),
            sim_require_finite=True, sim_require_nnan=True, nc=nc)
        return tuple(outs)

    try:
        devices = jax.devices("axon")[:N_CORES]
    except Exception:
        devices = jax.devices()[:N_CORES]
    mesh = Mesh(np.asarray(devices), ("core",))
    n_outs = len(out_names)
    fn = jax.jit(
        shard_map(_bodyfn, mesh=mesh,
                  in_specs=(PartitionSpec("core"),) * (n_params + n_outs),
                  out_specs=(PartitionSpec("core"),) * n_outs,
                  check_rep=False),
        keep_unused=True)
    sharding = jax.sharding.NamedSharding(mesh, PartitionSpec("core"))
    r = {"fn": fn, "in_names": in_names, "out_names": out_names,
         "zero_outs": zero_outs, "nc": nc, "sharding": sharding}
    _cache[key] = r
    return r


def _prep_host(x, W_enc, b_enc, W_dec, b_dec):
    x_eff = x - b_dec[None, :]
    xT_full = np.ascontiguousarray(x_eff.T, dtype=np.float32)      # [D, B]
    wencT = np.ascontiguousarray(W_enc.T, dtype=np.float32)        # [D, F]
    wdecT16 = np.ascontiguousarray(W_dec.T, dtype=np.float16)      # [F, D]
    beff = (b_enc.astype(np.float64)
            - W_enc.astype(np.float64) @ b_dec.astype(np.float64))
    beff = beff.astype(np.float32)[None, :]                        # [1, F]
    bdec = b_dec.astype(np.float32)[None, :]                       # [1, D]
    return xT_full, wencT, wdecT16, beff, bdec


def kernel(x, W_enc, b_enc, W_dec, b_dec, _repeat=1, _timeit=False):
    x = np.asarray(x, np.float32)
    W_enc = np.asarray(W_enc, np.float32)
    b_enc = np.asarray(b_enc, np.float32)
    W_dec = np.asarray(W_dec, np.float32)
    b_dec = np.asarray(b_dec, np.float32)
    xT_full, wencT, wdecT16, beff, bdec = _prep_host(x, W_enc, b_enc, W_dec, b_dec)
    has_benc = bool(np.any(beff))
    has_bdec = bool(np.any(b_dec))
    r = _get_runner(has_benc, has_bdec, _repeat)

    per_core = {
        "wdecT16": [wdecT16] * N_CORES,
        "beff": [beff] * N_CORES,
        "bdec": [bdec] * N_CORES,
    }
    if ENC_MODE == "bf16x3":
        import ml_dtypes
        bf16 = ml_dtypes.bfloat16
        xTh = xT_full.astype(bf16)
        xTl = (xT_full - xTh.astype(np.float32)).astype(bf16)
        wh = wencT.astype(bf16)
        wl = (wencT - wh.astype(np.float32)).astype(bf16)
        per_core["xTh"] = [np.ascontiguousarray(xTh[:, c * BC:(c + 1) * BC])
                           for c in range(N_CORES)]
        per_core["xTl"] = [np.ascontiguousarray(xTl[:, c * BC:(c + 1) * BC])
                           for c in range(N_CORES)]
        per_core["wencTh"] = [wh] * N_CORES
        per_core["wencTl"] = [wl] * N_CORES
    else:
        per_core["xT"] = [np.ascontiguousarray(xT_full[:, c * BC:(c + 1) * BC])
                          for c in range(N_CORES)]
        per_core["wencT"] = [wencT] * N_CORES
    args = [np.concatenate(per_core[name], axis=0) for name in r["in_names"]]
    args += [np.concatenate([z] * N_CORES, axis=0) for z in r["zero_outs"]]

    import jax, time
    dev_args = [jax.device_put(a, r["sharding"]) for a in args]
    kernel.last_dev_args = dev_args
    kernel.last_runner = r
    outs = r["fn"](*dev_args)
    jax.block_until_ready(outs)
    if _timeit:
        times = []
        for _ in range(_timeit if isinstance(_timeit, int) and _timeit > 1 else 8):
            t0 = time.perf_counter()
            outs = r["fn"](*dev_args)
            jax.block_until_ready(outs)
            times.append(time.perf_counter() - t0)
        kernel.last_times = times

    xhat = np.asarray(outs[r["out_names"].index("xhat")])  # [B, D] concat
    return xhat.astype(np.float32)
